# revision 8
# baseline (speedup 1.0000x reference)
"""PixelCrossAttention2D Trainium2 kernel.

Math (per sample b):
    xq = x @ W1 + b1            # [hw, F]
    yk = y @ W2 + b2            # [hw, F]
    A  = sigmoid(yk @ xq.T)     # [hw, hw]
    out = A @ yk + xq           # [hw, F]

Sharding: 8 cores = 4 samples x 2 row-halves. Core (b, h) computes output rows
[h*2048, (h+1)*2048) of sample b. Host rolls the sample's pixel axis by h*2048
so every core runs the identical SPMD program on rows 0:2048 (the j-sum over
all 4096 pixels is permutation invariant). Host also pre-transposes x/y to
feature-major [C, hw] (the layout every on-chip matmul wants) and transposes
the [F, 2048] per-core result back.

On-chip per core (all matmuls in float32r, full PE rate at N=512):
    xqT = W1.T @ xT + b1        # [F, 4096]   (lhsT=W1, rhs=xT)
    ykT = W2.T @ yT + b2        # [F, 4096]
    yk_nat[t] = PE-transpose(ykT tile t)      # [j, F] per 128-pixel tile
    for ch in {0, 1}:           # i-columns 1024 per chunk
      for jt in 0..31:
        qk_psum[j=128, i=1024] = xqT[:, jt].T @ ykT[:, ch]      # logits.T
        at = sigmoid(qk_psum)   # ACT, PSUM -> SBUF
        av_psum[F, 1024] += yk_nat[jt].T @ at                   # out.T
      outT[:, ch] = av_psum + xqT[:, ch]       # residual
"""

import numpy as np

import concourse.bass as bass
import concourse.mybir as mybir
import concourse.tile as tile
from concourse import bacc, bass_utils
from concourse.masks import make_identity

F32 = mybir.dt.float32
F32R = mybir.dt.float32r

N_CORES = 8
HW = 4096          # pixels per sample
NF = 128           # feature dim
I_ROWS = 2048      # output rows per core
I_CHUNK = 1024     # i-columns per PSUM chunk
N_JT = HW // 128   # 32 j-tiles

_CACHE = {}


def _build():
    nc = bacc.Bacc("TRN2", target_bir_lowering=False, debug=False,
                   num_devices=N_CORES)
    xT_d = nc.dram_tensor("xT", [128, HW], F32, kind="ExternalInput")
    yT_d = nc.dram_tensor("yT", [128, HW], F32, kind="ExternalInput")
    w1_d = nc.dram_tensor("W1", [128, NF], F32, kind="ExternalInput")
    w2_d = nc.dram_tensor("W2", [128, NF], F32, kind="ExternalInput")
    b1_d = nc.dram_tensor("b1", [128, 1], F32, kind="ExternalInput")
    b2_d = nc.dram_tensor("b2", [128, 1], F32, kind="ExternalInput")
    outT_d = nc.dram_tensor("outT", [128, I_ROWS], F32, kind="ExternalOutput")

    with tile.TileContext(nc) as tc:
        with (
            tc.tile_pool(name="const", bufs=1) as cp,
            tc.tile_pool(name="big", bufs=1) as bp,
            tc.tile_pool(name="ob", bufs=2) as ob,
            tc.tile_pool(name="at", bufs=3) as atp,
            tc.tile_pool(name="psq", bufs=2, space="PSUM") as psq,
            tc.tile_pool(name="psav", bufs=1, space="PSUM") as psav,
            tc.tile_pool(name="pst", bufs=2, space="PSUM") as pst,
        ):
            w1 = cp.tile([128, NF], F32, tag="w1")
            w2 = cp.tile([128, NF], F32, tag="w2")
            b1 = cp.tile([128, 1], F32, tag="b1")
            b2 = cp.tile([128, 1], F32, tag="b2")
            ident = cp.tile([128, 128], F32, tag="ident")
            identr = cp.tile([128, 128], F32R, tag="identr")
            sgdummy = cp.tile([128, 1], F32, tag="sgdummy")

            # preload the sigmoid ACT table while DMAs run
            nc.gpsimd.memset(sgdummy[:], 0.0)
            nc.scalar.activation(
                sgdummy[:], sgdummy[:], mybir.ActivationFunctionType.Sigmoid)

            nc.sync.dma_start(w1[:], w1_d.ap())
            nc.sync.dma_start(w2[:], w2_d.ap())
            nc.sync.dma_start(b1[:], b1_d.ap())
            nc.sync.dma_start(b2[:], b2_d.ap())
            make_identity(nc, ident[:])
            nc.vector.tensor_copy(identr[:], ident[:])

            xT = bp.tile([128, HW], F32, tag="xT")
            yT = bp.tile([128, HW], F32, tag="yT")
            xTr = bp.tile([128, HW], F32R, tag="xTr")
            yTr = bp.tile([128, HW], F32R, tag="yTr")
            xqT = bp.tile([128, HW], F32R, tag="xqT")
            ykT = bp.tile([128, HW], F32R, tag="ykT")
            w1r = cp.tile([128, NF], F32R, tag="w1r")
            w2r = cp.tile([128, NF], F32R, tag="w2r")
            nc.vector.tensor_copy(w1r[:], w1[:])
            nc.vector.tensor_copy(w2r[:], w2[:])

            yk_nat = bp.tile([128, N_JT, 128], F32R, tag="yk_nat")
            for c in range(4):
                sl = bass.ts(c, 1024)
                # DMA chunk c of both inputs
                nc.sync.dma_start(yT[:, sl], yT_d.ap()[:, sl])
                nc.sync.dma_start(xT[:, sl], xT_d.ap()[:, sl])
                # f32r rounding copies on GPSIMD (idle otherwise; SBUF->SBUF)
                nc.gpsimd.tensor_copy(yTr[:, sl], yT[:, sl])
                nc.gpsimd.tensor_copy(xTr[:, sl], xT[:, sl])
                # projections: ykT = W2.T @ yT + b2 first (feeds transposes)
                py = psq.tile([128, 1024], F32, tag="qk")
                for s in range(2):
                    ssl = bass.ds(c * 1024 + s * 512, 512)
                    nc.tensor.matmul(py[:, bass.ts(s, 512)],
                                     w2r[:], yTr[:, ssl],
                                     start=True, stop=True)
                nc.vector.tensor_scalar_add(ykT[:, sl], py[:], b2[:])
                px = psq.tile([128, 1024], F32, tag="qk")
                for s in range(2):
                    ssl = bass.ds(c * 1024 + s * 512, 512)
                    nc.tensor.matmul(px[:, bass.ts(s, 512)],
                                     w1r[:], xTr[:, ssl],
                                     start=True, stop=True)
                nc.vector.tensor_scalar_add(xqT[:, sl], px[:], b1[:])
                # yk pixel-major tiles for this chunk (8 transposes)
                for t in range(c * 8, c * 8 + 8):
                    pt = pst.tile([128, 128], F32R, tag="tr")
                    nc.tensor.transpose(
                        pt[:], ykT[:, bass.ts(t, 128)], identr[:])
                    nc.vector.tensor_copy(yk_nat[:, t, :], pt[:])

            # main loop
            for ch in range(I_ROWS // I_CHUNK):
                av = psav.tile([128, I_CHUNK], F32, tag="av")
                for jt in range(N_JT):
                    qk = psq.tile([128, I_CHUNK], F32, tag="qk")
                    lhs = xqT[:, bass.ts(jt, 128)]
                    for s in range(2):
                        sl = bass.ds(ch * I_CHUNK + s * 512, 512)
                        nc.tensor.matmul(
                            qk[:, bass.ts(s, 512)], lhs,
                            ykT[:, sl],
                            start=True, stop=True)
                    at = atp.tile([128, I_CHUNK], F32R, tag="at")
                    nc.scalar.activation(
                        at[:], qk[:], mybir.ActivationFunctionType.Sigmoid)
                    for s in range(2):
                        nc.tensor.matmul(
                            av[:, bass.ts(s, 512)],
                            yk_nat[:, jt, :],
                            at[:, bass.ts(s, 512)],
                            start=(jt == 0), stop=(jt == N_JT - 1))
                oT = ob.tile([128, I_CHUNK], F32, tag="oT")
                nc.vector.tensor_add(
                    oT[:], av[:], xqT[:, bass.ts(ch, I_CHUNK)].bitcast(F32))
                nc.sync.dma_start(outT_d.ap()[:, bass.ts(ch, I_CHUNK)], oT[:])

    nc.compile()
    return nc


def get_nc():
    if "nc" not in _CACHE:
        _CACHE["nc"] = _build()
    return _CACHE["nc"]


def make_in_maps(x, y, W1, b1, W2, b2):
    B, H, W, C = x.shape
    hw = H * W
    xf = np.ascontiguousarray(x, dtype=np.float32).reshape(B, hw, C)
    yf = np.ascontiguousarray(y, dtype=np.float32).reshape(B, hw, C)
    W1 = np.ascontiguousarray(W1, dtype=np.float32)
    W2 = np.ascontiguousarray(W2, dtype=np.float32)
    b1c = np.ascontiguousarray(b1, dtype=np.float32).reshape(128, 1)
    b2c = np.ascontiguousarray(b2, dtype=np.float32).reshape(128, 1)
    in_maps = []
    for core in range(N_CORES):
        b, h = divmod(core, 2)
        s = h * I_ROWS
        xr = np.roll(xf[b], -s, axis=0)
        yr = np.roll(yf[b], -s, axis=0)
        in_maps.append({
            "xT": np.ascontiguousarray(xr.T),
            "yT": np.ascontiguousarray(yr.T),
            "W1": W1, "W2": W2, "b1": b1c, "b2": b2c,
        })
    return in_maps


def run(inputs, trace=False):
    nc = get_nc()
    in_maps = make_in_maps(**inputs)
    res = bass_utils.run_bass_kernel_spmd(
        nc, in_maps, list(range(N_CORES)), trace=trace)
    x = inputs["x"]
    B, H, W, C = x.shape
    out = np.empty((B, H * W, NF), np.float32)
    for core in range(N_CORES):
        b, h = divmod(core, 2)
        out[b, h * I_ROWS:(h + 1) * I_ROWS, :] = res.results[core]["outT"].T
    return out.reshape(B, H, W, NF), res


def kernel(**inputs):
    out, _ = run(inputs, trace=False)
    return out


# revision 11
# speedup vs baseline: 1.1538x; 1.1538x over previous
"""PixelCrossAttention2D Trainium2 kernel.

Math (per sample b):
    xq = x @ W1 + b1            # [hw, F]
    yk = y @ W2 + b2            # [hw, F]
    A  = sigmoid(yk @ xq.T)     # [hw, hw]
    out = A @ yk + xq           # [hw, F]

Sharding: 8 cores = 4 samples x 2 row-halves. Core (b, h) computes output rows
[h*2048, (h+1)*2048) of sample b. Host rolls the sample's pixel axis by h*2048
so every core runs the identical SPMD program on rows 0:2048 (the j-sum over
all 4096 pixels is permutation invariant). Host also pre-transposes x/y to
feature-major [C, hw] (the layout every on-chip matmul wants) and transposes
the [F, 2048] per-core result back.

On-chip per core:
    xqT = W1.T @ xT + b1        # [F, 4096]   (lhsT=W1, rhs=xT)
    ykT = W2.T @ yT + b2        # [F, 4096]
    yk_nat[t] = PE-transpose(ykT tile t)      # [j, F] per 128-pixel tile
    for ch in {0, 1}:           # i-columns 1024 per chunk
      for jt in 0..31:
        qk_psum[j=128, i=1024] = xqT[:, jt].T @ ykT[:, ch]      # logits.T
        at = sigmoid(qk_psum)   # ACT, PSUM -> SBUF, bf16
        av_psum[F, 1024] += yk_nat[jt].T @ at                   # out.T
      outT[:, ch] = av_psum + xqT[:, ch]       # residual

QK_DT picks the QK/projection matmul dtype: bf16 streams 1 col per 2.4 GHz
cycle; float32r takes 2x but carries ~19 mantissa bits. AV always runs bf16
(A is in [0,1]; the f32 PSUM accumulation keeps the j-sum accurate).
"""

import numpy as np

import concourse.bass as bass
import concourse.mybir as mybir
import concourse.tile as tile
from concourse import bacc, bass_utils
from concourse.masks import make_identity

F32 = mybir.dt.float32
F32R = mybir.dt.float32r
BF16 = mybir.dt.bfloat16

QK_DT = BF16       # BF16 or F32R
WIDE_MM = False    # N=1024 matmul outputs cross PSUM banks -> must stay 512

N_CORES = 8
HW = 4096          # pixels per sample
NF = 128           # feature dim
I_ROWS = 2048      # output rows per core
I_CHUNK = 1024     # i-columns per PSUM chunk
N_JT = HW // 128   # 32 j-tiles

_CACHE = {}


def _build():
    nc = bacc.Bacc("TRN2", target_bir_lowering=False, debug=False,
                   num_devices=N_CORES)
    xT_d = nc.dram_tensor("xT", [128, HW], F32, kind="ExternalInput")
    yT_d = nc.dram_tensor("yT", [128, HW], F32, kind="ExternalInput")
    w1_d = nc.dram_tensor("W1", [128, NF], F32, kind="ExternalInput")
    w2_d = nc.dram_tensor("W2", [128, NF], F32, kind="ExternalInput")
    b1_d = nc.dram_tensor("b1", [128, 1], F32, kind="ExternalInput")
    b2_d = nc.dram_tensor("b2", [128, 1], F32, kind="ExternalInput")
    outT_d = nc.dram_tensor("outT", [128, I_ROWS], F32, kind="ExternalOutput")

    # rounding-cast engine: Pool is fine for bf16; float32r needs DVE/ACT
    def rcast(dst, src):
        if QK_DT == BF16:
            nc.gpsimd.tensor_copy(dst, src)
        else:
            nc.vector.tensor_copy(dst, src)

    qk_wide = WIDE_MM and QK_DT == BF16

    with tile.TileContext(nc) as tc:
        with (
            tc.tile_pool(name="const", bufs=1) as cp,
            tc.tile_pool(name="big", bufs=1) as bp,
            tc.tile_pool(name="ob", bufs=2) as ob,
            tc.tile_pool(name="at", bufs=4) as atp,
            tc.tile_pool(name="psq", bufs=3, space="PSUM") as psq,
        ):
            w1 = cp.tile([128, NF], F32, tag="w1")
            w2 = cp.tile([128, NF], F32, tag="w2")
            b1 = cp.tile([128, 1], F32, tag="b1")
            b2 = cp.tile([128, 1], F32, tag="b2")
            ident = cp.tile([128, 128], F32, tag="ident")
            identq = cp.tile([128, 128], QK_DT, tag="identq")
            sgdummy = cp.tile([128, 1], F32, tag="sgdummy")

            # preload the sigmoid ACT table while DMAs run
            nc.gpsimd.memset(sgdummy[:], 0.0)
            nc.scalar.activation(
                sgdummy[:], sgdummy[:], mybir.ActivationFunctionType.Sigmoid)

            nc.sync.dma_start(w1[:], w1_d.ap())
            nc.sync.dma_start(w2[:], w2_d.ap())
            nc.sync.dma_start(b1[:], b1_d.ap())
            nc.sync.dma_start(b2[:], b2_d.ap())
            make_identity(nc, ident[:])
            nc.vector.tensor_copy(identq[:], ident[:])

            xT = bp.tile([128, HW], F32, tag="xT")
            yT = bp.tile([128, HW], F32, tag="yT")
            xTq = bp.tile([128, HW], QK_DT, tag="xTq")
            yTq = bp.tile([128, HW], QK_DT, tag="yTq")
            xqT = bp.tile([128, HW], QK_DT, tag="xqT")
            ykT = bp.tile([128, HW], QK_DT, tag="ykT")
            w1q = cp.tile([128, NF], QK_DT, tag="w1q")
            w2q = cp.tile([128, NF], QK_DT, tag="w2q")
            nc.vector.tensor_copy(w1q[:], w1[:])
            nc.vector.tensor_copy(w2q[:], w2[:])

            yk_nat = bp.tile([128, N_JT, 128], BF16, tag="yk_nat")
            for c in range(4):
                sl = bass.ts(c, 1024)
                # DMA chunk c of both inputs
                nc.sync.dma_start(yT[:, sl], yT_d.ap()[:, sl])
                nc.sync.dma_start(xT[:, sl], xT_d.ap()[:, sl])
                # rounding casts for matmul operands
                rcast(yTq[:, sl], yT[:, sl])
                rcast(xTq[:, sl], xT[:, sl])
                # projections: ykT = W2.T @ yT + b2 first (feeds transposes)
                py = psq.tile([128, 1024], F32, tag="qk")
                if qk_wide:
                    nc.tensor.matmul(py[:], w2q[:], yTq[:, sl],
                                     start=True, stop=True)
                else:
                    for s in range(2):
                        ssl = bass.ds(c * 1024 + s * 512, 512)
                        nc.tensor.matmul(py[:, bass.ts(s, 512)],
                                         w2q[:], yTq[:, ssl],
                                         start=True, stop=True)
                nc.vector.tensor_scalar_add(ykT[:, sl], py[:], b2[:])
                px = psq.tile([128, 1024], F32, tag="qk")
                if qk_wide:
                    nc.tensor.matmul(px[:], w1q[:], xTq[:, sl],
                                     start=True, stop=True)
                else:
                    for s in range(2):
                        ssl = bass.ds(c * 1024 + s * 512, 512)
                        nc.tensor.matmul(px[:, bass.ts(s, 512)],
                                         w1q[:], xTq[:, ssl],
                                         start=True, stop=True)
                nc.vector.tensor_scalar_add(xqT[:, sl], px[:], b1[:])
                # yk pixel-major tiles for this chunk (8 transposes)
                for t in range(c * 8, c * 8 + 8):
                    pt = pst.tile([128, 128], QK_DT, tag="tr")
                    nc.tensor.transpose(
                        pt[:], ykT[:, bass.ts(t, 128)], identq[:])
                    nc.vector.tensor_copy(yk_nat[:, t, :], pt[:])

            # main loop
            for ch in range(I_ROWS // I_CHUNK):
                av = psav.tile([128, I_CHUNK], F32, tag="av")
                for jt in range(N_JT):
                    qk = psq.tile([128, I_CHUNK], F32, tag="qk")
                    lhs = xqT[:, bass.ts(jt, 128)]
                    if qk_wide:
                        nc.tensor.matmul(
                            qk[:], lhs, ykT[:, bass.ts(ch, I_CHUNK)],
                            start=True, stop=True)
                    else:
                        for s in range(2):
                            sl = bass.ds(ch * I_CHUNK + s * 512, 512)
                            nc.tensor.matmul(
                                qk[:, bass.ts(s, 512)], lhs, ykT[:, sl],
                                start=True, stop=True)
                    at = atp.tile([128, I_CHUNK], BF16, tag="at")
                    nc.scalar.activation(
                        at[:], qk[:], mybir.ActivationFunctionType.Sigmoid)
                    if WIDE_MM:
                        nc.tensor.matmul(
                            av[:], yk_nat[:, jt, :], at[:],
                            start=(jt == 0), stop=(jt == N_JT - 1))
                    else:
                        for s in range(2):
                            nc.tensor.matmul(
                                av[:, bass.ts(s, 512)],
                                yk_nat[:, jt, :], at[:, bass.ts(s, 512)],
                                start=(jt == 0), stop=(jt == N_JT - 1))
                oT = ob.tile([128, I_CHUNK], F32, tag="oT")
                res = xqT[:, bass.ts(ch, I_CHUNK)]
                if QK_DT == F32R:
                    res = res.bitcast(F32)
                nc.vector.tensor_add(oT[:], av[:], res)
                nc.sync.dma_start(outT_d.ap()[:, bass.ts(ch, I_CHUNK)], oT[:])

    nc.compile()
    return nc


def get_nc():
    if "nc" not in _CACHE:
        _CACHE["nc"] = _build()
    return _CACHE["nc"]


def make_in_maps(x, y, W1, b1, W2, b2):
    B, H, W, C = x.shape
    hw = H * W
    xf = np.ascontiguousarray(x, dtype=np.float32).reshape(B, hw, C)
    yf = np.ascontiguousarray(y, dtype=np.float32).reshape(B, hw, C)
    W1 = np.ascontiguousarray(W1, dtype=np.float32)
    W2 = np.ascontiguousarray(W2, dtype=np.float32)
    b1c = np.ascontiguousarray(b1, dtype=np.float32).reshape(128, 1)
    b2c = np.ascontiguousarray(b2, dtype=np.float32).reshape(128, 1)
    in_maps = []
    for core in range(N_CORES):
        b, h = divmod(core, 2)
        s = h * I_ROWS
        xr = np.roll(xf[b], -s, axis=0)
        yr = np.roll(yf[b], -s, axis=0)
        in_maps.append({
            "xT": np.ascontiguousarray(xr.T),
            "yT": np.ascontiguousarray(yr.T),
            "W1": W1, "W2": W2, "b1": b1c, "b2": b2c,
        })
    return in_maps


def run(inputs, trace=False):
    nc = get_nc()
    in_maps = make_in_maps(**inputs)
    res = bass_utils.run_bass_kernel_spmd(
        nc, in_maps, list(range(N_CORES)), trace=trace)
    x = inputs["x"]
    B, H, W, C = x.shape
    out = np.empty((B, H * W, NF), np.float32)
    for core in range(N_CORES):
        b, h = divmod(core, 2)
        out[b, h * I_ROWS:(h + 1) * I_ROWS, :] = res.results[core]["outT"].T
    return out.reshape(B, H, W, NF), res


def kernel(**inputs):
    out, _ = run(inputs, trace=False)
    return out


# revision 17
# speedup vs baseline: 1.4469x; 1.2540x over previous
"""PixelCrossAttention2D Trainium2 kernel.

Math (per sample b):
    xq = x @ W1 + b1            # [hw, F]
    yk = y @ W2 + b2            # [hw, F]
    A  = sigmoid(yk @ xq.T)     # [hw, hw]
    out = A @ yk + xq           # [hw, F]

Sharding: 8 cores = 4 samples x 2 row-halves. Core (b, h) computes output rows
[h*2048, (h+1)*2048) of sample b. Host rolls the sample's pixel axis by h*2048
so every core runs the identical SPMD program on rows 0:2048 (the j-sum over
all 4096 pixels is permutation invariant). Host also pre-transposes x/y to
feature-major [C, hw] (the layout every on-chip matmul wants) and transposes
the [F, 2048] per-core result back.

On-chip per core:
    xqT = W1.T @ xT + b1        # [F, 4096]   (lhsT=W1, rhs=xT)
    ykT = W2.T @ yT + b2        # [F, 4096]
    yk_nat[t] = PE-transpose(ykT tile t)      # [j, F] per 128-pixel tile
    for ch in {0, 1}:           # i-columns 1024 per chunk
      for jt in 0..31:
        qk_psum[j=128, i=1024] = xqT[:, jt].T @ ykT[:, ch]      # logits.T
        at = sigmoid(qk_psum)   # ACT, PSUM -> SBUF, bf16
        av_psum[F, 1024] += yk_nat[jt].T @ at                   # out.T
      outT[:, ch] = av_psum + xqT[:, ch]       # residual

QK_DT picks the QK/projection matmul dtype: bf16 streams 1 col per 2.4 GHz
cycle; float32r takes 2x but carries ~19 mantissa bits. AV always runs bf16
(A is in [0,1]; the f32 PSUM accumulation keeps the j-sum accurate).
"""

import numpy as np

import concourse.bass as bass
import concourse.mybir as mybir
import concourse.tile as tile
from concourse import bacc, bass_utils
from concourse.masks import make_identity

F32 = mybir.dt.float32
F32R = mybir.dt.float32r
BF16 = mybir.dt.bfloat16

QK_DT = BF16       # BF16 or F32R
WIDE_MM = False    # N=1024 matmul outputs cross PSUM banks -> must stay 512

N_CORES = 8
HW = 4096          # pixels per sample
NF = 128           # feature dim
I_ROWS = 2048      # output rows per core
I_CHUNK = 1024     # i-columns per PSUM chunk
N_JT = HW // 128   # 32 j-tiles

_CACHE = {}


def _build():
    nc = bacc.Bacc("TRN2", target_bir_lowering=False, debug=False,
                   num_devices=N_CORES)
    # bf16: host pre-casts x/y (bf16 needs no on-chip rounding producer);
    # float32r: f32 inputs + on-chip DVE rounding copies (verifier rule)
    in_dt = BF16 if QK_DT == BF16 else F32
    xT_d = nc.dram_tensor("xT", [128, HW], in_dt, kind="ExternalInput")
    yT_d = nc.dram_tensor("yT", [128, HW], in_dt, kind="ExternalInput")
    w1_d = nc.dram_tensor("W1", [128, NF], F32, kind="ExternalInput")
    w2_d = nc.dram_tensor("W2", [128, NF], F32, kind="ExternalInput")
    b1_d = nc.dram_tensor("b1", [128, 1], F32, kind="ExternalInput")
    b2_d = nc.dram_tensor("b2", [128, 1], F32, kind="ExternalInput")
    outT_d = nc.dram_tensor("outT", [128, I_ROWS], F32, kind="ExternalOutput")

    qk_wide = WIDE_MM and QK_DT == BF16

    with tile.TileContext(nc) as tc:
        with (
            tc.tile_pool(name="const", bufs=1) as cp,
            tc.tile_pool(name="big", bufs=1) as bp,
            tc.tile_pool(name="ob", bufs=2) as ob,
            tc.tile_pool(name="at", bufs=4) as atp,
            tc.tile_pool(name="psq", bufs=3, space="PSUM") as psq,
        ):
            w1 = cp.tile([128, NF], F32, tag="w1")
            w2 = cp.tile([128, NF], F32, tag="w2")
            b1 = cp.tile([128, 1], F32, tag="b1")
            b2 = cp.tile([128, 1], F32, tag="b2")
            ident = cp.tile([128, 128], F32, tag="ident")
            identq = cp.tile([128, 128], QK_DT, tag="identq")
            sgdummy = cp.tile([128, 1], F32, tag="sgdummy")

            # preload the sigmoid ACT table while DMAs run
            nc.gpsimd.memset(sgdummy[:], 0.0)
            nc.scalar.activation(
                sgdummy[:], sgdummy[:], mybir.ActivationFunctionType.Sigmoid)

            nc.sync.dma_start(w1[:], w1_d.ap())
            nc.sync.dma_start(w2[:], w2_d.ap())
            nc.sync.dma_start(b1[:], b1_d.ap())
            nc.sync.dma_start(b2[:], b2_d.ap())
            make_identity(nc, ident[:])
            nc.vector.tensor_copy(identq[:], ident[:])

            if QK_DT == BF16:
                xTq = bp.tile([128, HW], BF16, tag="xTq")
                yTq = bp.tile([128, HW], BF16, tag="yTq")
            else:
                xT = bp.tile([128, HW], F32, tag="xT")
                yT = bp.tile([128, HW], F32, tag="yT")
                xTq = bp.tile([128, HW], QK_DT, tag="xTq")
                yTq = bp.tile([128, HW], QK_DT, tag="yTq")
            xqT = bp.tile([128, HW], QK_DT, tag="xqT")
            ykT = bp.tile([128, HW], QK_DT, tag="ykT")
            w1q = cp.tile([128, NF], QK_DT, tag="w1q")
            w2q = cp.tile([128, NF], QK_DT, tag="w2q")
            nc.vector.tensor_copy(w1q[:], w1[:])
            nc.vector.tensor_copy(w2q[:], w2[:])

            yk_nat = bp.tile([128, N_JT, 128], BF16, tag="yk_nat")
            pst_cm = tc.tile_pool(name="pst", bufs=2, space="PSUM")
            pst = pst_cm.__enter__()
            for c in range(4):
                sl = bass.ts(c, 1024)
                # DMA chunk c of both inputs (+ DVE rounding for float32r)
                if QK_DT == BF16:
                    nc.sync.dma_start(yTq[:, sl], yT_d.ap()[:, sl])
                    nc.sync.dma_start(xTq[:, sl], xT_d.ap()[:, sl])
                else:
                    nc.sync.dma_start(yT[:, sl], yT_d.ap()[:, sl])
                    nc.sync.dma_start(xT[:, sl], xT_d.ap()[:, sl])
                    nc.vector.tensor_copy(yTq[:, sl], yT[:, sl])
                    nc.vector.tensor_copy(xTq[:, sl], xT[:, sl])
                # projections: ykT = W2.T @ yT + b2 first (feeds transposes)
                py = psq.tile([128, 1024], F32, tag="qk")
                if qk_wide:
                    nc.tensor.matmul(py[:], w2q[:], yTq[:, sl],
                                     start=True, stop=True)
                else:
                    for s in range(2):
                        ssl = bass.ds(c * 1024 + s * 512, 512)
                        nc.tensor.matmul(py[:, bass.ts(s, 512)],
                                         w2q[:], yTq[:, ssl],
                                         start=True, stop=True)
                nc.vector.tensor_scalar_add(ykT[:, sl], py[:], b2[:])
                px = psq.tile([128, 1024], F32, tag="qk")
                if qk_wide:
                    nc.tensor.matmul(px[:], w1q[:], xTq[:, sl],
                                     start=True, stop=True)
                else:
                    for s in range(2):
                        ssl = bass.ds(c * 1024 + s * 512, 512)
                        nc.tensor.matmul(px[:, bass.ts(s, 512)],
                                         w1q[:], xTq[:, ssl],
                                         start=True, stop=True)
                nc.vector.tensor_scalar_add(xqT[:, sl], px[:], b1[:])
                # yk pixel-major tiles for this chunk (8 transposes)
                for t in range(c * 8, c * 8 + 8):
                    pt = pst.tile([128, 128], QK_DT, tag="tr")
                    nc.tensor.transpose(
                        pt[:], ykT[:, bass.ts(t, 128)], identq[:])
                    nc.vector.tensor_copy(yk_nat[:, t, :], pt[:])

            # transpose PSUM pool closes -> its banks go to the AV accumulator
            pst_cm.__exit__(None, None, None)

            # main loop
            with tc.tile_pool(name="psav", bufs=1, space="PSUM") as psav:
                for ch in range(I_ROWS // I_CHUNK):
                    av = psav.tile([128, I_CHUNK], F32, tag="av")
                    for jt in range(N_JT):
                        qk = psq.tile([128, I_CHUNK], F32, tag="qk")
                        lhs = xqT[:, bass.ts(jt, 128)]
                        if qk_wide:
                            nc.tensor.matmul(
                                qk[:], lhs, ykT[:, bass.ts(ch, I_CHUNK)],
                                start=True, stop=True)
                        else:
                            for s in range(2):
                                sl = bass.ds(ch * I_CHUNK + s * 512, 512)
                                nc.tensor.matmul(
                                    qk[:, bass.ts(s, 512)], lhs, ykT[:, sl],
                                    start=True, stop=True)
                        at = atp.tile([128, I_CHUNK], BF16, tag="at")
                        nc.scalar.activation(
                            at[:], qk[:], mybir.ActivationFunctionType.Sigmoid)
                        if WIDE_MM:
                            nc.tensor.matmul(
                                av[:], yk_nat[:, jt, :], at[:],
                                start=(jt == 0), stop=(jt == N_JT - 1))
                        else:
                            for s in range(2):
                                nc.tensor.matmul(
                                    av[:, bass.ts(s, 512)],
                                    yk_nat[:, jt, :], at[:, bass.ts(s, 512)],
                                    start=(jt == 0), stop=(jt == N_JT - 1))
                    oT = ob.tile([128, I_CHUNK], F32, tag="oT")
                    res = xqT[:, bass.ts(ch, I_CHUNK)]
                    if QK_DT == F32R:
                        res = res.bitcast(F32)
                    nc.vector.tensor_add(oT[:], av[:], res)
                    nc.sync.dma_start(
                        outT_d.ap()[:, bass.ts(ch, I_CHUNK)], oT[:])

    nc.compile()
    return nc


def get_nc():
    if "nc" not in _CACHE:
        _CACHE["nc"] = _build()
    return _CACHE["nc"]


def make_in_maps(x, y, W1, b1, W2, b2):
    B, H, W, C = x.shape
    hw = H * W
    xf = np.ascontiguousarray(x, dtype=np.float32).reshape(B, hw, C)
    yf = np.ascontiguousarray(y, dtype=np.float32).reshape(B, hw, C)
    W1 = np.ascontiguousarray(W1, dtype=np.float32)
    W2 = np.ascontiguousarray(W2, dtype=np.float32)
    b1c = np.ascontiguousarray(b1, dtype=np.float32).reshape(128, 1)
    b2c = np.ascontiguousarray(b2, dtype=np.float32).reshape(128, 1)
    import ml_dtypes
    in_np = ml_dtypes.bfloat16 if QK_DT == BF16 else np.float32
    in_maps = []
    for core in range(N_CORES):
        b, h = divmod(core, 2)
        s = h * I_ROWS
        xr = np.roll(xf[b], -s, axis=0)
        yr = np.roll(yf[b], -s, axis=0)
        in_maps.append({
            "xT": np.ascontiguousarray(xr.T).astype(in_np),
            "yT": np.ascontiguousarray(yr.T).astype(in_np),
            "W1": W1, "W2": W2, "b1": b1c, "b2": b2c,
        })
    return in_maps


def run(inputs, trace=False):
    nc = get_nc()
    in_maps = make_in_maps(**inputs)
    res = bass_utils.run_bass_kernel_spmd(
        nc, in_maps, list(range(N_CORES)), trace=trace)
    x = inputs["x"]
    B, H, W, C = x.shape
    out = np.empty((B, H * W, NF), np.float32)
    for core in range(N_CORES):
        b, h = divmod(core, 2)
        out[b, h * I_ROWS:(h + 1) * I_ROWS, :] = res.results[core]["outT"].T
    return out.reshape(B, H, W, NF), res


def kernel(**inputs):
    out, _ = run(inputs, trace=False)
    return out
